# revision 2
# baseline (speedup 1.0000x reference)
"""LSTM autoencoder (4 stacked Keras-style LSTM layers, relu cell activation)
on 8 Trainium2 NeuronCores.

Strategy: data-parallel over batch (B=64 -> 8 per core). On each core the 4
layers are processed as a chunk-lagged wavefront: layer l works on time-chunk
(phase - l) while layer l+1 works on the previous chunk, so the 4 per-step
recurrence dependency chains interleave and hide each other's latency.
Input projections (x @ W + b) are done in bulk per chunk on the tensor engine;
the per-timestep recurrence matmul z^T = U^T @ h^T keeps U as the stationary
operand so the output lands transposed ([4H, Bc] folded into [128, ...]),
which keeps every elementwise gate op on all 128 partitions.

v2: all matmul operands in bf16 (4x fewer PE cycles/row vs fp32, and FWL
weight loads are 2x faster for non-fp32); gates/zx/h in bf16 (DVE 2x/4x
modes); h state lives directly in the TC+1-deep hist buffer so the
per-step h copy is gone. c stays fp32.

Host-side (untimed) prep: batch shard, transpose x to [128, KF, T, Bc],
permute gate columns from reference order [i,f,g,o] to [i,f,o,g] so sigmoid
gates are contiguous, fold weight K-tiles into partition-major layout.
"""

import sys

sys.path.insert(0, "/opt/trn_rl_repo")

import numpy as np
import ml_dtypes

import concourse.bass as bass
import concourse.bacc as bacc
import concourse.mybir as mybir
import concourse.tile as tile
from concourse.bass_utils import run_bass_kernel_spmd

F32 = mybir.dt.float32
BF16 = mybir.dt.bfloat16
AF = mybir.ActivationFunctionType

B, T, INPUT_LEN = 64, 1024, 256
NCORES = 8
BC = B // NCORES  # batch per core = 8
TC = 64  # timesteps per chunk
NCH = T // TC  # 16 chunks
NL = 4
# (in_features, hidden) per layer
LAYERS = [(256, 256), (256, 128), (128, 256), (256, 256)]
NPH = NCH + NL - 1  # wavefront phases
UNROLL = 4

_CACHE = {}


def _gate_perm(h):
    # reference gate order in the 4H axis: i, f, g, o  ->  ours: i, f, o, g
    return np.concatenate(
        [np.arange(0, h), np.arange(h, 2 * h), np.arange(3 * h, 4 * h), np.arange(2 * h, 3 * h)]
    )


def _fold_w(w):
    # [K, N] -> [128, (K//128) * N] with K-tiles side by side (partition major)
    k, n = w.shape
    kt = k // 128
    return np.ascontiguousarray(w.reshape(kt, 128, n).transpose(1, 0, 2).reshape(128, kt * n))


def _build():
    nc = bacc.Bacc("TRN2", target_bir_lowering=False, debug=False, num_devices=NCORES)

    xT_d = nc.dram_tensor("xT", [128, 2, T, BC], BF16, kind="ExternalInput")
    out_d = nc.dram_tensor("outT", [128, 2, T, BC], BF16, kind="ExternalOutput")
    w_d, u_d, b_d = [], [], []
    for li, (f, h) in enumerate(LAYERS):
        kf, kh, m = f // 128, h // 128, 4 * h // 128
        w_d.append(nc.dram_tensor(f"W{li}", [128, kf * 4 * h], BF16, kind="ExternalInput"))
        u_d.append(nc.dram_tensor(f"U{li}", [128, kh * 4 * h], BF16, kind="ExternalInput"))
        b_d.append(nc.dram_tensor(f"b{li}", [128, m], F32, kind="ExternalInput"))

    with tile.TileContext(nc) as tc:
        with (
            tc.tile_pool(name="const", bufs=1) as cpool,
            tc.tile_pool(name="state", bufs=1) as spool,
            tc.tile_pool(name="xin", bufs=2) as xpool,
            tc.tile_pool(name="zpsum", bufs=1, space="PSUM") as zpp,
            tc.tile_pool(name="ipsum", bufs=2, space="PSUM") as ipp,
        ):
            w_sb, u_sb, b_sb = [], [], []
            zx_sb, hist_sb = [], []
            c_st, z_sb, g_sb, t1_sb, t2_sb, zps = [], [], [], [], [], []
            for li, (f, h) in enumerate(LAYERS):
                kf, kh, m = f // 128, h // 128, 4 * h // 128
                w_sb.append(cpool.tile([128, kf * 4 * h], BF16, tag=f"w{li}", name=f"w{li}"))
                u_sb.append(cpool.tile([128, kh * 4 * h], BF16, tag=f"u{li}", name=f"u{li}"))
                b_sb.append(cpool.tile([128, m], F32, tag=f"b{li}", name=f"b{li}"))
                nc.sync.dma_start(w_sb[li][:], w_d[li][:])
                nc.sync.dma_start(u_sb[li][:], u_d[li][:])
                nc.sync.dma_start(b_sb[li][:], b_d[li][:])
                zx_sb.append(spool.tile([128, m, TC, BC], BF16, tag=f"zx{li}", name=f"zx{li}"))
                # hist has TC+1 slots: slot 0 carries h_{-1} (prev chunk's last h)
                hist_sb.append(
                    spool.tile([128, kh, TC + 1, BC], BF16, tag=f"hist{li}", name=f"hist{li}")
                )
                c_st.append(spool.tile([128, kh, 1, BC], F32, tag=f"c{li}", name=f"c{li}"))
                z_sb.append(spool.tile([128, m, 1, BC], BF16, tag=f"z{li}", name=f"z{li}"))
                g_sb.append(spool.tile([128, m, 1, BC], BF16, tag=f"g{li}", name=f"g{li}"))
                t1_sb.append(spool.tile([128, kh, 1, BC], BF16, tag=f"t1{li}", name=f"t1{li}"))
                t2_sb.append(spool.tile([128, kh, 1, BC], F32, tag=f"t2{li}", name=f"t2{li}"))
                zps.append(zpp.tile([128, m, 1, BC], F32, tag=f"zp{li}", name=f"zp{li}"))

            def superstep(li, iv):
                f, h = LAYERS[li]
                kh, m = h // 128, 4 * h // 128
                fh = 4 * h
                zp, zs, gs, cc = zps[li], z_sb[li], g_sb[li], c_st[li]
                hist = hist_sb[li]
                for mi in range(m):
                    for k in range(kh):
                        nc.tensor.matmul(
                            zp[:, mi, :, :],
                            u_sb[li][:, k * fh + mi * 128 : k * fh + (mi + 1) * 128],
                            hist[:, k, bass.ds(iv, 1), :],
                            start=(k == 0),
                            stop=(k == kh - 1),
                        )
                zxs = zx_sb[li][:, :, bass.ds(iv, 1), :]
                nc.vector.tensor_add(zs[:], zp[:], zxs)
                nc.scalar.activation(gs[:, 0 : 3 * kh], zs[:, 0 : 3 * kh], AF.Sigmoid)
                nc.scalar.activation(gs[:, 3 * kh : 4 * kh], zs[:, 3 * kh : 4 * kh], AF.Relu)
                i_g = gs[:, 0:kh]
                f_g = gs[:, kh : 2 * kh]
                o_g = gs[:, 2 * kh : 3 * kh]
                g_g = gs[:, 3 * kh : 4 * kh]
                nc.vector.tensor_mul(t1_sb[li][:], i_g, g_g)
                nc.vector.tensor_mul(t2_sb[li][:], f_g, cc[:])
                nc.vector.tensor_add(cc[:], t1_sb[li][:], t2_sb[li][:])
                nc.vector.tensor_mul(t2_sb[li][:], o_g, cc[:])
                # o > 0 always, so h = o * relu(c) = relu(o * c); write h into hist slot iv+1
                nc.vector.tensor_scalar_max(
                    hist[:, :, bass.ds(iv + 1, 1), :], t2_sb[li][:], 0.0
                )

            def inproj(li, src):
                # zx_l = W_l^T @ src + b_l for a whole chunk; src [128, kf, TC, BC]
                f, h = LAYERS[li]
                kf, m = f // 128, 4 * h // 128
                fh = 4 * h
                for mi in range(m):
                    ps = ipp.tile([128, TC, BC], F32, tag="ip", name="ip")
                    for k in range(kf):
                        nc.tensor.matmul(
                            ps[:],
                            w_sb[li][:, k * fh + mi * 128 : k * fh + (mi + 1) * 128],
                            src[:, k, :, :],
                            start=(k == 0),
                            stop=(k == kf - 1),
                        )
                    nc.scalar.activation(
                        zx_sb[li][:, mi, :, :], ps[:], AF.Identity, bias=b_sb[li][:, mi : mi + 1]
                    )

            for p in range(NPH):
                active = [li for li in range(NL) if 0 <= p - li < NCH]
                for li in active:
                    c = p - li
                    if li == 0:
                        xt = xpool.tile([128, 2, TC, BC], BF16, tag="xt", name="xt")
                        nc.sync.dma_start(xt[:], xT_d[:, :, c * TC : (c + 1) * TC, :])
                        inproj(0, xt)
                    else:
                        inproj(li, hist_sb[li - 1][:, :, 1 : TC + 1, :])
                    if c == 0:
                        nc.gpsimd.memset(hist_sb[li][:, :, 0:1, :], 0.0)
                        nc.gpsimd.memset(c_st[li][:], 0.0)
                    else:
                        # carry h state: last slot of prev chunk -> slot 0
                        nc.vector.tensor_copy(
                            hist_sb[li][:, :, 0:1, :], hist_sb[li][:, :, TC : TC + 1, :]
                        )
                with tc.For_i(0, TC, UNROLL) as iv:
                    for u in range(UNROLL):
                        for li in active:
                            superstep(li, iv + u)
                if NL - 1 in active:
                    c4 = p - (NL - 1)
                    nc.sync.dma_start(
                        out_d[:, :, c4 * TC : (c4 + 1) * TC, :],
                        hist_sb[NL - 1][:, :, 1 : TC + 1, :],
                    )
    nc.compile()
    return nc


def _prep_inputs(x, ws, us, bs):
    base = {}
    for li, (f, h) in enumerate(LAYERS):
        perm = _gate_perm(h)
        base[f"W{li}"] = _fold_w(ws[li][:, perm]).astype(ml_dtypes.bfloat16)
        base[f"U{li}"] = _fold_w(us[li][:, perm]).astype(ml_dtypes.bfloat16)
        bb = bs[li][perm]
        base[f"b{li}"] = np.ascontiguousarray(bb.reshape(4 * h // 128, 128).T)  # [128, m]

    in_maps = []
    for ci in range(NCORES):
        xc = x[ci * BC : (ci + 1) * BC]  # [BC, T, F]
        xT = np.ascontiguousarray(
            xc.reshape(BC, T, 2, 128).transpose(3, 2, 1, 0)
        ).astype(ml_dtypes.bfloat16)  # [128, 2, T, BC]
        m = dict(base)
        m["xT"] = xT
        in_maps.append(m)
    return in_maps


def kernel(x, W1, U1, b1, W2, U2, b2, W3, U3, b3, W4, U4, b4):
    x = np.asarray(x, dtype=np.float32)
    ws = [np.asarray(a, np.float32) for a in (W1, W2, W3, W4)]
    us = [np.asarray(a, np.float32) for a in (U1, U2, U3, U4)]
    bs = [np.asarray(a, np.float32) for a in (b1, b2, b3, b4)]

    if "nc" not in _CACHE:
        _CACHE["nc"] = _build()
    nc = _CACHE["nc"]

    in_maps = _prep_inputs(x, ws, us, bs)
    _CACHE["last_in_maps"] = in_maps

    res = run_bass_kernel_spmd(nc, in_maps, list(range(NCORES)))
    outs = []
    for ci in range(NCORES):
        oT = np.asarray(res.results[ci]["outT"], dtype=np.float32)  # [128, 2, T, BC]
        outs.append(np.ascontiguousarray(oT.transpose(3, 2, 1, 0).reshape(BC, T, 256)))
    return np.concatenate(outs, axis=0)


# revision 3
# speedup vs baseline: 1.1863x; 1.1863x over previous
"""V5d: data-parallel wavefront, two-group combined elementwise.

On top of V5's statically-addressed recurrence matmuls (27ns/MM bursts), the
gate elementwise is combined across layer groups {L1,L3} and {L2,L4} to cut
the DVE op count per step from ~24 tiny ops to 4 z-adds + 2x6 wide ops,
while keeping two independent chains so the engines stagger. c state bf16.
Gate relu runs on DVE (off Act), h history copies on GpSimd (off-chain).
"""

import sys

sys.path.insert(0, "/opt/trn_rl_repo")

import numpy as np
import ml_dtypes

import concourse.bass as bass
import concourse.bacc as bacc
import concourse.mybir as mybir
import concourse.tile as tile
import time as _time
from concourse.bass_utils import run_bass_kernel_spmd

F32 = mybir.dt.float32
BF16 = mybir.dt.bfloat16
AF = mybir.ActivationFunctionType

B, T, INPUT_LEN = 64, 1024, 256
NCORES = 8
BC = B // NCORES
TC = 64
NCH = T // TC
NL = 4
LAYERS = [(256, 256), (256, 128), (128, 256), (256, 256)]
KHS = [h // 128 for _, h in LAYERS]
NPH = NCH + NL - 1
UNROLL = 4

# two chain groups: layers {0, 2} and {1, 3}
GROUPS = [[0, 2], [1, 3]]
# offset of each layer's kh tiles within its group's tile dim
GOFF = {}
GNT = []
for gi, g in enumerate(GROUPS):
    o = 0
    for li in g:
        GOFF[li] = (gi, o)
        o += KHS[li]
    GNT.append(o)

_CACHE = {}


def _gate_perm(h):
    return np.concatenate(
        [np.arange(0, h), np.arange(h, 2 * h), np.arange(3 * h, 4 * h), np.arange(2 * h, 3 * h)]
    )


def _fold_w(w):
    k, n = w.shape
    kt = k // 128
    return np.ascontiguousarray(w.reshape(kt, 128, n).transpose(1, 0, 2).reshape(128, kt * n))


def _build():
    nc = bacc.Bacc("TRN2", target_bir_lowering=False, debug=False, num_devices=NCORES)

    xT_d = nc.dram_tensor("xT", [128, 2, T, BC], BF16, kind="ExternalInput")
    out_d = nc.dram_tensor("outT", [128, 2, T, BC], BF16, kind="ExternalOutput")
    w_d, u_d, b_d = [], [], []
    for li, (f, h) in enumerate(LAYERS):
        kf, kh, m = f // 128, h // 128, 4 * h // 128
        w_d.append(nc.dram_tensor(f"W{li}", [128, kf * 4 * h], BF16, kind="ExternalInput"))
        u_d.append(nc.dram_tensor(f"U{li}", [128, kh * 4 * h], BF16, kind="ExternalInput"))
        b_d.append(nc.dram_tensor(f"b{li}", [128, m], F32, kind="ExternalInput"))

    with tile.TileContext(nc) as tc:
        with (
            tc.tile_pool(name="const", bufs=1) as cpool,
            tc.tile_pool(name="state", bufs=1) as spool,
            tc.tile_pool(name="xin", bufs=2) as xpool,
            tc.tile_pool(name="zpsum", bufs=1, space="PSUM") as zpp,
            tc.tile_pool(name="ipsum", bufs=2, space="PSUM") as ipp,
        ):
            w_sb, u_sb, b_sb, zx_sb, hist_sb, zps = [], [], [], [], [], []
            for li, (f, h) in enumerate(LAYERS):
                kf, kh, m = f // 128, h // 128, 4 * h // 128
                w_sb.append(cpool.tile([128, kf * 4 * h], BF16, tag=f"w{li}", name=f"w{li}"))
                u_sb.append(cpool.tile([128, kh * 4 * h], BF16, tag=f"u{li}", name=f"u{li}"))
                b_sb.append(cpool.tile([128, m], F32, tag=f"b{li}", name=f"b{li}"))
                nc.sync.dma_start(w_sb[li][:], w_d[li][:])
                nc.sync.dma_start(u_sb[li][:], u_d[li][:])
                nc.sync.dma_start(b_sb[li][:], b_d[li][:])
                zx_sb.append(
                    spool.tile([128, 4, kh, TC, BC], BF16, tag=f"zx{li}", name=f"zx{li}")
                )
                hist_sb.append(
                    spool.tile([128, kh, TC, BC], BF16, tag=f"hist{li}", name=f"hist{li}")
                )
                zps.append(zpp.tile([128, 4, kh, 1, BC], F32, tag=f"zp{li}", name=f"zp{li}"))

            # group-combined tiles: [128, kind(4), nt, 1, BC]
            z_g, g_g, c_g, t1_g, t2_g, hc_g = [], [], [], [], [], []
            for gi in range(2):
                nt = GNT[gi]
                z_g.append(spool.tile([128, 4, nt, 1, BC], BF16, tag=f"zg{gi}", name=f"zg{gi}"))
                g_g.append(spool.tile([128, 4, nt, 1, BC], BF16, tag=f"gg{gi}", name=f"gg{gi}"))
                c_g.append(spool.tile([128, nt, 1, BC], BF16, tag=f"cg{gi}", name=f"cg{gi}"))
                t1_g.append(spool.tile([128, nt, 1, BC], BF16, tag=f"t1g{gi}", name=f"t1g{gi}"))
                t2_g.append(spool.tile([128, nt, 1, BC], BF16, tag=f"t2g{gi}", name=f"t2g{gi}"))
                # parity-buffered current h per group (static matmul source)
                hc_g.append(
                    spool.tile([128, 2, nt, 1, BC], BF16, tag=f"hcg{gi}", name=f"hcg{gi}")
                )
            for li in range(NL):
                nc.vector.memset(zps[li][:], 0.0)
            for gi in range(2):
                nc.vector.memset(hc_g[gi][:], 0.0)
                nc.vector.memset(c_g[gi][:], 0.0)

            def step_mms(li, par):
                f, h = LAYERS[li]
                kh = h // 128
                fh = 4 * h
                gi, o = GOFF[li]
                for mi in range(4 * kh):
                    kind, j = mi // kh, mi % kh
                    for k in range(kh):
                        nc.tensor.matmul(
                            zps[li][:, kind, j, :, :],
                            u_sb[li][:, k * fh + mi * 128 : k * fh + (mi + 1) * 128],
                            hc_g[gi][:, par, o + k, :, :],
                            start=(k == 0),
                            stop=(k == kh - 1),
                        )

            def step_elem(active, iv, par):
                # z = zp + zx into the group tile, per layer (separate PSUM banks)
                for li in active:
                    kh = KHS[li]
                    gi, o = GOFF[li]
                    nc.vector.tensor_add(
                        z_g[gi][:, :, o : o + kh, :, :],
                        zps[li][:],
                        zx_sb[li][:, :, :, bass.ds(iv, 1), :],
                    )
                for gi in range(2):
                    nc.scalar.activation(g_g[gi][:, 0:3], z_g[gi][:, 0:3], AF.Sigmoid)
                    nc.vector.tensor_scalar_max(g_g[gi][:, 3], z_g[gi][:, 3], 0.0)
                for gi in range(2):
                    i_k, f_k, o_k, gk = (
                        g_g[gi][:, 0],
                        g_g[gi][:, 1],
                        g_g[gi][:, 2],
                        g_g[gi][:, 3],
                    )
                    nc.vector.tensor_mul(t1_g[gi][:], i_k, gk)
                    nc.vector.tensor_mul(t2_g[gi][:], f_k, c_g[gi][:])
                    nc.vector.tensor_add(c_g[gi][:], t1_g[gi][:], t2_g[gi][:])
                    nc.vector.tensor_mul(t2_g[gi][:], o_k, c_g[gi][:])
                    nc.vector.tensor_scalar_max(hc_g[gi][:, 1 - par], t2_g[gi][:], 0.0)
                # off-chain: record h into per-layer history
                for li in active:
                    kh = KHS[li]
                    gi, o = GOFF[li]
                    nc.gpsimd.tensor_copy(
                        hist_sb[li][:, :, bass.ds(iv, 1), :],
                        hc_g[gi][:, 1 - par, o : o + kh, :, :],
                    )

            def inproj(li, src):
                f, h = LAYERS[li]
                kf, kh, m = f // 128, h // 128, 4 * h // 128
                fh = 4 * h
                for mi in range(m):
                    kind, j = mi // kh, mi % kh
                    ps = ipp.tile([128, TC, BC], F32, tag="ip", name="ip")
                    for k in range(kf):
                        nc.tensor.matmul(
                            ps[:],
                            w_sb[li][:, k * fh + mi * 128 : k * fh + (mi + 1) * 128],
                            src[:, k, :, :],
                            start=(k == 0),
                            stop=(k == kf - 1),
                        )
                    nc.scalar.activation(
                        zx_sb[li][:, kind, j, :, :],
                        ps[:],
                        AF.Identity,
                        bias=b_sb[li][:, mi : mi + 1],
                    )

            for p in range(NPH):
                active = [li for li in range(NL) if 0 <= p - li < NCH]
                for li in active:
                    c = p - li
                    if li == 0:
                        xt = xpool.tile([128, 2, TC, BC], BF16, tag="xt", name="xt")
                        nc.sync.dma_start(xt[:], xT_d[:, :, c * TC : (c + 1) * TC, :])
                        inproj(0, xt)
                    else:
                        inproj(li, hist_sb[li - 1])
                    if c == 0:
                        gi, o = GOFF[li]
                        kh = KHS[li]
                        nc.vector.memset(hc_g[gi][:, 0, o : o + kh, :, :], 0.0)
                        nc.vector.memset(c_g[gi][:, o : o + kh, :, :], 0.0)
                with tc.For_i(0, TC, UNROLL) as iv:
                    for u in range(UNROLL):
                        for li in active:
                            step_mms(li, u % 2)
                        step_elem(active, iv + u, u % 2)
                if NL - 1 in active:
                    c4 = p - (NL - 1)
                    nc.sync.dma_start(
                        out_d[:, :, c4 * TC : (c4 + 1) * TC, :], hist_sb[NL - 1][:]
                    )
    nc.compile()
    return nc


def _prep_inputs(x, ws, us, bs):
    base = {}
    for li, (f, h) in enumerate(LAYERS):
        perm = _gate_perm(h)
        base[f"W{li}"] = _fold_w(ws[li][:, perm]).astype(ml_dtypes.bfloat16)
        base[f"U{li}"] = _fold_w(us[li][:, perm]).astype(ml_dtypes.bfloat16)
        bb = bs[li][perm]
        base[f"b{li}"] = np.ascontiguousarray(bb.reshape(4 * h // 128, 128).T)

    in_maps = []
    for ci in range(NCORES):
        xc = x[ci * BC : (ci + 1) * BC]
        xT = np.ascontiguousarray(xc.reshape(BC, T, 2, 128).transpose(3, 2, 1, 0)).astype(
            ml_dtypes.bfloat16
        )
        m = dict(base)
        m["xT"] = xT
        in_maps.append(m)
    return in_maps


def kernel(x, W1, U1, b1, W2, U2, b2, W3, U3, b3, W4, U4, b4):
    x = np.asarray(x, dtype=np.float32)
    ws = [np.asarray(a, np.float32) for a in (W1, W2, W3, W4)]
    us = [np.asarray(a, np.float32) for a in (U1, U2, U3, U4)]
    bs = [np.asarray(a, np.float32) for a in (b1, b2, b3, b4)]

    if "nc" not in _CACHE:
        _CACHE["nc"] = _build()
    nc = _CACHE["nc"]

    in_maps = _prep_inputs(x, ws, us, bs)
    _CACHE["last_in_maps"] = in_maps

    res = None
    last_err = None
    for _attempt in range(3):
        try:
            res = run_bass_kernel_spmd(nc, in_maps, list(range(NCORES)))
            break
        except Exception as e:  # transient device-unrecoverable reports
            last_err = e
            _time.sleep(5)
    if res is None:
        raise last_err
    outs = []
    for ci in range(NCORES):
        oT = np.asarray(res.results[ci]["outT"], dtype=np.float32)
        outs.append(np.ascontiguousarray(oT.transpose(3, 2, 1, 0).reshape(BC, T, 256)))
    return np.concatenate(outs, axis=0)


# revision 4
# speedup vs baseline: 1.2653x; 1.0666x over previous
"""V5d: data-parallel wavefront, two-group combined elementwise.

On top of V5's statically-addressed recurrence matmuls (27ns/MM bursts), the
gate elementwise is combined across layer groups {L1,L3} and {L2,L4} to cut
the DVE op count per step from ~24 tiny ops to 4 z-adds + 2x6 wide ops,
while keeping two independent chains so the engines stagger. c state bf16.
Gate relu runs on DVE (off Act), h history copies on GpSimd (off-chain).
"""

import sys

sys.path.insert(0, "/opt/trn_rl_repo")

import numpy as np
import ml_dtypes

import concourse.bass as bass
import concourse.bacc as bacc
import concourse.mybir as mybir
import concourse.tile as tile
import time as _time
from concourse.bass_utils import run_bass_kernel_spmd

F32 = mybir.dt.float32
BF16 = mybir.dt.bfloat16
AF = mybir.ActivationFunctionType

B, T, INPUT_LEN = 64, 1024, 256
NCORES = 8
BC = B // NCORES
TC = 64
NCH = T // TC
NL = 4
LAYERS = [(256, 256), (256, 128), (128, 256), (256, 256)]
KHS = [h // 128 for _, h in LAYERS]
NPH = NCH + NL - 1
UNROLL = 8

# two chain groups: layers {0, 2} and {1, 3}
GROUPS = [[0, 2], [1, 3]]
# offset of each layer's kh tiles within its group's tile dim
GOFF = {}
GNT = []
for gi, g in enumerate(GROUPS):
    o = 0
    for li in g:
        GOFF[li] = (gi, o)
        o += KHS[li]
    GNT.append(o)

_CACHE = {}


def _gate_perm(h):
    return np.concatenate(
        [np.arange(0, h), np.arange(h, 2 * h), np.arange(3 * h, 4 * h), np.arange(2 * h, 3 * h)]
    )


def _fold_w(w):
    k, n = w.shape
    kt = k // 128
    return np.ascontiguousarray(w.reshape(kt, 128, n).transpose(1, 0, 2).reshape(128, kt * n))


def _build():
    nc = bacc.Bacc("TRN2", target_bir_lowering=False, debug=False, num_devices=NCORES)

    xT_d = nc.dram_tensor("xT", [128, 2, T, BC], BF16, kind="ExternalInput")
    out_d = nc.dram_tensor("outT", [128, 2, T, BC], BF16, kind="ExternalOutput")
    w_d, u_d, b_d = [], [], []
    for li, (f, h) in enumerate(LAYERS):
        kf, kh, m = f // 128, h // 128, 4 * h // 128
        w_d.append(nc.dram_tensor(f"W{li}", [128, kf * 4 * h], BF16, kind="ExternalInput"))
        u_d.append(nc.dram_tensor(f"U{li}", [128, kh * 4 * h], BF16, kind="ExternalInput"))
        b_d.append(nc.dram_tensor(f"b{li}", [128, m], F32, kind="ExternalInput"))

    with tile.TileContext(nc) as tc:
        with (
            tc.tile_pool(name="const", bufs=1) as cpool,
            tc.tile_pool(name="state", bufs=1) as spool,
            tc.tile_pool(name="xin", bufs=2) as xpool,
            tc.tile_pool(name="zpsum", bufs=1, space="PSUM") as zpp,
            tc.tile_pool(name="ipsum", bufs=2, space="PSUM") as ipp,
        ):
            w_sb, u_sb, b_sb, zx_sb, hist_sb, zps = [], [], [], [], [], []
            for li, (f, h) in enumerate(LAYERS):
                kf, kh, m = f // 128, h // 128, 4 * h // 128
                w_sb.append(cpool.tile([128, kf * 4 * h], BF16, tag=f"w{li}", name=f"w{li}"))
                u_sb.append(cpool.tile([128, kh * 4 * h], BF16, tag=f"u{li}", name=f"u{li}"))
                b_sb.append(cpool.tile([128, m], F32, tag=f"b{li}", name=f"b{li}"))
                nc.sync.dma_start(w_sb[li][:], w_d[li][:])
                nc.sync.dma_start(u_sb[li][:], u_d[li][:])
                nc.sync.dma_start(b_sb[li][:], b_d[li][:])
                zx_sb.append(
                    spool.tile([128, 4, kh, TC, BC], BF16, tag=f"zx{li}", name=f"zx{li}")
                )
                hist_sb.append(
                    spool.tile([128, kh, TC, BC], BF16, tag=f"hist{li}", name=f"hist{li}")
                )
                nb = 2 if li in (0, 2) else 1
                zps.append(
                    [
                        zpp.tile([128, 4, kh, 1, BC], F32, tag=f"zp{li}_{q}", name=f"zp{li}_{q}")
                        for q in range(nb)
                    ]
                )

            # group-combined tiles: [128, kind(4), nt, 1, BC]
            z_g, g_g, c_g, t1_g, t2_g, hc_g = [], [], [], [], [], []
            for gi in range(2):
                nt = GNT[gi]
                z_g.append(spool.tile([128, 4, nt, 1, BC], BF16, tag=f"zg{gi}", name=f"zg{gi}"))
                g_g.append(spool.tile([128, 4, nt, 1, BC], BF16, tag=f"gg{gi}", name=f"gg{gi}"))
                c_g.append(spool.tile([128, nt, 1, BC], BF16, tag=f"cg{gi}", name=f"cg{gi}"))
                t1_g.append(spool.tile([128, nt, 1, BC], BF16, tag=f"t1g{gi}", name=f"t1g{gi}"))
                t2_g.append(spool.tile([128, nt, 1, BC], BF16, tag=f"t2g{gi}", name=f"t2g{gi}"))
                # parity-buffered current h per group (static matmul source)
                hc_g.append(
                    spool.tile([128, 2, nt, 1, BC], BF16, tag=f"hcg{gi}", name=f"hcg{gi}")
                )
            for li in range(NL):
                for t in zps[li]:
                    nc.vector.memset(t[:], 0.0)
            for gi in range(2):
                nc.vector.memset(hc_g[gi][:], 0.0)
                nc.vector.memset(c_g[gi][:], 0.0)

            def step_mms(li, par):
                f, h = LAYERS[li]
                kh = h // 128
                fh = 4 * h
                gi, o = GOFF[li]
                zt = zps[li][par % len(zps[li])]
                for mi in range(4 * kh):
                    kind, j = mi // kh, mi % kh
                    for k in range(kh):
                        nc.tensor.matmul(
                            zt[:, kind, j, :, :],
                            u_sb[li][:, k * fh + mi * 128 : k * fh + (mi + 1) * 128],
                            hc_g[gi][:, par, o + k, :, :],
                            start=(k == 0),
                            stop=(k == kh - 1),
                        )

            def step_elem(active, iv, par):
                # z = zp + zx into the group tile, per layer (separate PSUM banks)
                for li in active:
                    kh = KHS[li]
                    gi, o = GOFF[li]
                    nc.vector.tensor_add(
                        z_g[gi][:, :, o : o + kh, :, :],
                        zps[li][par % len(zps[li])][:],
                        zx_sb[li][:, :, :, bass.ds(iv, 1), :],
                    )
                for gi in range(2):
                    nc.scalar.activation(g_g[gi][:, 0:3], z_g[gi][:, 0:3], AF.Sigmoid)
                    nc.vector.tensor_scalar_max(g_g[gi][:, 3], z_g[gi][:, 3], 0.0)
                for gi in range(2):
                    i_k, f_k, o_k, gk = (
                        g_g[gi][:, 0],
                        g_g[gi][:, 1],
                        g_g[gi][:, 2],
                        g_g[gi][:, 3],
                    )
                    nc.vector.tensor_mul(t1_g[gi][:], i_k, gk)
                    nc.vector.tensor_mul(t2_g[gi][:], f_k, c_g[gi][:])
                    nc.vector.tensor_add(c_g[gi][:], t1_g[gi][:], t2_g[gi][:])
                    nc.vector.tensor_mul(t2_g[gi][:], o_k, c_g[gi][:])
                    nc.vector.tensor_scalar_max(hc_g[gi][:, 1 - par], t2_g[gi][:], 0.0)
                # off-chain: record h into per-layer history
                for li in active:
                    kh = KHS[li]
                    gi, o = GOFF[li]
                    nc.gpsimd.tensor_copy(
                        hist_sb[li][:, :, bass.ds(iv, 1), :],
                        hc_g[gi][:, 1 - par, o : o + kh, :, :],
                    )

            def inproj(li, src):
                f, h = LAYERS[li]
                kf, kh, m = f // 128, h // 128, 4 * h // 128
                fh = 4 * h
                for mi in range(m):
                    kind, j = mi // kh, mi % kh
                    ps = ipp.tile([128, TC, BC], F32, tag="ip", name="ip")
                    for k in range(kf):
                        nc.tensor.matmul(
                            ps[:],
                            w_sb[li][:, k * fh + mi * 128 : k * fh + (mi + 1) * 128],
                            src[:, k, :, :],
                            start=(k == 0),
                            stop=(k == kf - 1),
                        )
                    nc.scalar.activation(
                        zx_sb[li][:, kind, j, :, :],
                        ps[:],
                        AF.Identity,
                        bias=b_sb[li][:, mi : mi + 1],
                    )

            for p in range(NPH):
                active = [li for li in range(NL) if 0 <= p - li < NCH]
                for li in active:
                    c = p - li
                    if li == 0:
                        xt = xpool.tile([128, 2, TC, BC], BF16, tag="xt", name="xt")
                        nc.sync.dma_start(xt[:], xT_d[:, :, c * TC : (c + 1) * TC, :])
                        inproj(0, xt)
                    else:
                        inproj(li, hist_sb[li - 1])
                    if c == 0:
                        gi, o = GOFF[li]
                        kh = KHS[li]
                        nc.vector.memset(hc_g[gi][:, 0, o : o + kh, :, :], 0.0)
                        nc.vector.memset(c_g[gi][:, o : o + kh, :, :], 0.0)
                with tc.For_i(0, TC, UNROLL) as iv:
                    for u in range(UNROLL):
                        for li in active:
                            step_mms(li, u % 2)
                        step_elem(active, iv + u, u % 2)
                if NL - 1 in active:
                    c4 = p - (NL - 1)
                    nc.sync.dma_start(
                        out_d[:, :, c4 * TC : (c4 + 1) * TC, :], hist_sb[NL - 1][:]
                    )
    nc.compile()
    return nc


def _prep_inputs(x, ws, us, bs):
    base = {}
    for li, (f, h) in enumerate(LAYERS):
        perm = _gate_perm(h)
        base[f"W{li}"] = _fold_w(ws[li][:, perm]).astype(ml_dtypes.bfloat16)
        base[f"U{li}"] = _fold_w(us[li][:, perm]).astype(ml_dtypes.bfloat16)
        bb = bs[li][perm]
        base[f"b{li}"] = np.ascontiguousarray(bb.reshape(4 * h // 128, 128).T)

    in_maps = []
    for ci in range(NCORES):
        xc = x[ci * BC : (ci + 1) * BC]
        xT = np.ascontiguousarray(xc.reshape(BC, T, 2, 128).transpose(3, 2, 1, 0)).astype(
            ml_dtypes.bfloat16
        )
        m = dict(base)
        m["xT"] = xT
        in_maps.append(m)
    return in_maps


def kernel(x, W1, U1, b1, W2, U2, b2, W3, U3, b3, W4, U4, b4):
    x = np.asarray(x, dtype=np.float32)
    ws = [np.asarray(a, np.float32) for a in (W1, W2, W3, W4)]
    us = [np.asarray(a, np.float32) for a in (U1, U2, U3, U4)]
    bs = [np.asarray(a, np.float32) for a in (b1, b2, b3, b4)]

    if "nc" not in _CACHE:
        _CACHE["nc"] = _build()
    nc = _CACHE["nc"]

    in_maps = _prep_inputs(x, ws, us, bs)
    _CACHE["last_in_maps"] = in_maps

    res = None
    last_err = None
    for _attempt in range(3):
        try:
            res = run_bass_kernel_spmd(nc, in_maps, list(range(NCORES)))
            break
        except Exception as e:  # transient device-unrecoverable reports
            last_err = e
            _time.sleep(5)
    if res is None:
        raise last_err
    outs = []
    for ci in range(NCORES):
        oT = np.asarray(res.results[ci]["outT"], dtype=np.float32)
        outs.append(np.ascontiguousarray(oT.transpose(3, 2, 1, 0).reshape(BC, T, 256)))
    return np.concatenate(outs, axis=0)


# revision 5
# speedup vs baseline: 1.2673x; 1.0016x over previous
"""V5d: data-parallel wavefront, two-group combined elementwise.

On top of V5's statically-addressed recurrence matmuls (27ns/MM bursts), the
gate elementwise is combined across layer groups {L1,L3} and {L2,L4} to cut
the DVE op count per step from ~24 tiny ops to 4 z-adds + 2x6 wide ops,
while keeping two independent chains so the engines stagger. c state bf16.
Gate relu runs on DVE (off Act), h history copies on GpSimd (off-chain).
"""

import sys

sys.path.insert(0, "/opt/trn_rl_repo")

import numpy as np
import ml_dtypes

import concourse.bass as bass
import concourse.bacc as bacc
import concourse.mybir as mybir
import concourse.tile as tile
import time as _time
from concourse.bass_utils import run_bass_kernel_spmd

F32 = mybir.dt.float32
BF16 = mybir.dt.bfloat16
AF = mybir.ActivationFunctionType

B, T, INPUT_LEN = 64, 1024, 256
NCORES = 8
BC = B // NCORES
TC = 64
NCH = T // TC
NL = 4
LAYERS = [(256, 256), (256, 128), (128, 256), (256, 256)]
KHS = [h // 128 for _, h in LAYERS]
NPH = NCH + NL - 1
UNROLL = 16

# two chain groups: layers {0, 2} and {1, 3}
GROUPS = [[0, 2], [1, 3]]
# offset of each layer's kh tiles within its group's tile dim
GOFF = {}
GNT = []
for gi, g in enumerate(GROUPS):
    o = 0
    for li in g:
        GOFF[li] = (gi, o)
        o += KHS[li]
    GNT.append(o)

_CACHE = {}


def _gate_perm(h):
    return np.concatenate(
        [np.arange(0, h), np.arange(h, 2 * h), np.arange(3 * h, 4 * h), np.arange(2 * h, 3 * h)]
    )


def _fold_w(w):
    k, n = w.shape
    kt = k // 128
    return np.ascontiguousarray(w.reshape(kt, 128, n).transpose(1, 0, 2).reshape(128, kt * n))


def _build():
    nc = bacc.Bacc("TRN2", target_bir_lowering=False, debug=False, num_devices=NCORES)

    xT_d = nc.dram_tensor("xT", [128, 2, T, BC], BF16, kind="ExternalInput")
    out_d = nc.dram_tensor("outT", [128, 2, T, BC], BF16, kind="ExternalOutput")
    w_d, u_d, b_d = [], [], []
    for li, (f, h) in enumerate(LAYERS):
        kf, kh, m = f // 128, h // 128, 4 * h // 128
        w_d.append(nc.dram_tensor(f"W{li}", [128, kf * 4 * h], BF16, kind="ExternalInput"))
        u_d.append(nc.dram_tensor(f"U{li}", [128, kh * 4 * h], BF16, kind="ExternalInput"))
        b_d.append(nc.dram_tensor(f"b{li}", [128, m], F32, kind="ExternalInput"))

    with tile.TileContext(nc) as tc:
        with (
            tc.tile_pool(name="const", bufs=1) as cpool,
            tc.tile_pool(name="state", bufs=1) as spool,
            tc.tile_pool(name="xin", bufs=2) as xpool,
            tc.tile_pool(name="zpsum", bufs=1, space="PSUM") as zpp,
            tc.tile_pool(name="ipsum", bufs=2, space="PSUM") as ipp,
        ):
            w_sb, u_sb, b_sb, zx_sb, hist_sb, zps = [], [], [], [], [], []
            for li, (f, h) in enumerate(LAYERS):
                kf, kh, m = f // 128, h // 128, 4 * h // 128
                w_sb.append(cpool.tile([128, kf * 4 * h], BF16, tag=f"w{li}", name=f"w{li}"))
                u_sb.append(cpool.tile([128, kh * 4 * h], BF16, tag=f"u{li}", name=f"u{li}"))
                b_sb.append(cpool.tile([128, m], F32, tag=f"b{li}", name=f"b{li}"))
                nc.sync.dma_start(w_sb[li][:], w_d[li][:])
                nc.sync.dma_start(u_sb[li][:], u_d[li][:])
                nc.sync.dma_start(b_sb[li][:], b_d[li][:])
                zx_sb.append(
                    spool.tile([128, 4, kh, TC, BC], BF16, tag=f"zx{li}", name=f"zx{li}")
                )
                hist_sb.append(
                    spool.tile([128, kh, TC, BC], BF16, tag=f"hist{li}", name=f"hist{li}")
                )
                nb = 2 if li in (0, 2) else 1
                zps.append(
                    [
                        zpp.tile([128, 4, kh, 1, BC], F32, tag=f"zp{li}_{q}", name=f"zp{li}_{q}")
                        for q in range(nb)
                    ]
                )

            # group-combined tiles: [128, kind(4), nt, 1, BC]
            z_g, g_g, c_g, t1_g, t2_g, hc_g = [], [], [], [], [], []
            for gi in range(2):
                nt = GNT[gi]
                z_g.append(spool.tile([128, 4, nt, 1, BC], BF16, tag=f"zg{gi}", name=f"zg{gi}"))
                g_g.append(spool.tile([128, 4, nt, 1, BC], BF16, tag=f"gg{gi}", name=f"gg{gi}"))
                c_g.append(spool.tile([128, nt, 1, BC], BF16, tag=f"cg{gi}", name=f"cg{gi}"))
                t1_g.append(spool.tile([128, nt, 1, BC], BF16, tag=f"t1g{gi}", name=f"t1g{gi}"))
                t2_g.append(spool.tile([128, nt, 1, BC], BF16, tag=f"t2g{gi}", name=f"t2g{gi}"))
                # parity-buffered current h per group (static matmul source)
                hc_g.append(
                    spool.tile([128, 2, nt, 1, BC], BF16, tag=f"hcg{gi}", name=f"hcg{gi}")
                )
            for li in range(NL):
                for t in zps[li]:
                    nc.vector.memset(t[:], 0.0)
            for gi in range(2):
                nc.vector.memset(hc_g[gi][:], 0.0)
                nc.vector.memset(c_g[gi][:], 0.0)

            def step_mms(li, par):
                f, h = LAYERS[li]
                kh = h // 128
                fh = 4 * h
                gi, o = GOFF[li]
                zt = zps[li][par % len(zps[li])]
                for mi in range(4 * kh):
                    kind, j = mi // kh, mi % kh
                    for k in range(kh):
                        nc.tensor.matmul(
                            zt[:, kind, j, :, :],
                            u_sb[li][:, k * fh + mi * 128 : k * fh + (mi + 1) * 128],
                            hc_g[gi][:, par, o + k, :, :],
                            start=(k == 0),
                            stop=(k == kh - 1),
                        )

            def z_add(li, iv, par):
                kh = KHS[li]
                gi, o = GOFF[li]
                nc.vector.tensor_add(
                    z_g[gi][:, :, o : o + kh, :, :],
                    zps[li][par % len(zps[li])][:],
                    zx_sb[li][:, :, :, bass.ds(iv, 1), :],
                )

            def step_elem(active, iv, par):
                for gi in range(2):
                    nc.scalar.activation(g_g[gi][:, 0:3], z_g[gi][:, 0:3], AF.Sigmoid)
                    nc.vector.tensor_scalar_max(g_g[gi][:, 3], z_g[gi][:, 3], 0.0)
                for gi in range(2):
                    i_k, f_k, o_k, gk = (
                        g_g[gi][:, 0],
                        g_g[gi][:, 1],
                        g_g[gi][:, 2],
                        g_g[gi][:, 3],
                    )
                    nc.vector.tensor_mul(t1_g[gi][:], i_k, gk)
                    nc.vector.tensor_mul(t2_g[gi][:], f_k, c_g[gi][:])
                    nc.vector.tensor_add(c_g[gi][:], t1_g[gi][:], t2_g[gi][:])
                    nc.vector.tensor_mul(t2_g[gi][:], o_k, c_g[gi][:])
                    nc.vector.tensor_scalar_max(hc_g[gi][:, 1 - par], t2_g[gi][:], 0.0)
                # off-chain: record h into per-layer history
                for li in active:
                    kh = KHS[li]
                    gi, o = GOFF[li]
                    nc.gpsimd.tensor_copy(
                        hist_sb[li][:, :, bass.ds(iv, 1), :],
                        hc_g[gi][:, 1 - par, o : o + kh, :, :],
                    )

            def inproj(li, src):
                f, h = LAYERS[li]
                kf, kh, m = f // 128, h // 128, 4 * h // 128
                fh = 4 * h
                for mi in range(m):
                    kind, j = mi // kh, mi % kh
                    ps = ipp.tile([128, TC, BC], F32, tag="ip", name="ip")
                    for k in range(kf):
                        nc.tensor.matmul(
                            ps[:],
                            w_sb[li][:, k * fh + mi * 128 : k * fh + (mi + 1) * 128],
                            src[:, k, :, :],
                            start=(k == 0),
                            stop=(k == kf - 1),
                        )
                    nc.scalar.activation(
                        zx_sb[li][:, kind, j, :, :],
                        ps[:],
                        AF.Identity,
                        bias=b_sb[li][:, mi : mi + 1],
                    )

            for p in range(NPH):
                active = [li for li in range(NL) if 0 <= p - li < NCH]
                for li in active:
                    c = p - li
                    if li == 0:
                        xt = xpool.tile([128, 2, TC, BC], BF16, tag="xt", name="xt")
                        nc.sync.dma_start(xt[:], xT_d[:, :, c * TC : (c + 1) * TC, :])
                        inproj(0, xt)
                    else:
                        inproj(li, hist_sb[li - 1])
                    if c == 0:
                        gi, o = GOFF[li]
                        kh = KHS[li]
                        nc.vector.memset(hc_g[gi][:, 0, o : o + kh, :, :], 0.0)
                        nc.vector.memset(c_g[gi][:, o : o + kh, :, :], 0.0)
                with tc.For_i(0, TC, UNROLL) as iv:
                    for u in range(UNROLL):
                        for li in active:
                            step_mms(li, u % 2)
                            z_add(li, iv + u, u % 2)
                        step_elem(active, iv + u, u % 2)
                if NL - 1 in active:
                    c4 = p - (NL - 1)
                    nc.sync.dma_start(
                        out_d[:, :, c4 * TC : (c4 + 1) * TC, :], hist_sb[NL - 1][:]
                    )
    nc.compile()
    return nc


def _prep_inputs(x, ws, us, bs):
    base = {}
    for li, (f, h) in enumerate(LAYERS):
        perm = _gate_perm(h)
        base[f"W{li}"] = _fold_w(ws[li][:, perm]).astype(ml_dtypes.bfloat16)
        base[f"U{li}"] = _fold_w(us[li][:, perm]).astype(ml_dtypes.bfloat16)
        bb = bs[li][perm]
        base[f"b{li}"] = np.ascontiguousarray(bb.reshape(4 * h // 128, 128).T)

    in_maps = []
    for ci in range(NCORES):
        xc = x[ci * BC : (ci + 1) * BC]
        xT = np.ascontiguousarray(xc.reshape(BC, T, 2, 128).transpose(3, 2, 1, 0)).astype(
            ml_dtypes.bfloat16
        )
        m = dict(base)
        m["xT"] = xT
        in_maps.append(m)
    return in_maps


def kernel(x, W1, U1, b1, W2, U2, b2, W3, U3, b3, W4, U4, b4):
    x = np.asarray(x, dtype=np.float32)
    ws = [np.asarray(a, np.float32) for a in (W1, W2, W3, W4)]
    us = [np.asarray(a, np.float32) for a in (U1, U2, U3, U4)]
    bs = [np.asarray(a, np.float32) for a in (b1, b2, b3, b4)]

    if "nc" not in _CACHE:
        _CACHE["nc"] = _build()
    nc = _CACHE["nc"]

    in_maps = _prep_inputs(x, ws, us, bs)
    _CACHE["last_in_maps"] = in_maps

    res = None
    last_err = None
    for _attempt in range(3):
        try:
            res = run_bass_kernel_spmd(nc, in_maps, list(range(NCORES)))
            break
        except Exception as e:  # transient device-unrecoverable reports
            last_err = e
            _time.sleep(5)
    if res is None:
        raise last_err
    outs = []
    for ci in range(NCORES):
        oT = np.asarray(res.results[ci]["outT"], dtype=np.float32)
        outs.append(np.ascontiguousarray(oT.transpose(3, 2, 1, 0).reshape(BC, T, 256)))
    return np.concatenate(outs, axis=0)
